# revision 1
# baseline (speedup 1.0000x reference)
"""3x3 grayscale dilation (all-ones SE) = 3x3 max-pool, stride 1, zero padding.

Input (8, 3, 1024, 1024) f32 -> same-shape output.
Sharding: 24 (B*C) images, 3 per NeuronCore across 8 cores.

Per-image layout: one [128, 8192] SBUF tile; partition p holds image rows
8p..8p+7 contiguously, so row neighbors are free-dim offsets of +-1024
except at partition boundaries.

Measured facts this design is built on:
  - All max work must run on DVE (TRN2 Pool engine rejects TensorTensor /
    Pool instructions; fp32 TT is 1 elem/cycle, cost ~ free-dim size,
    independent of partition count). Each 3-max stage uses the
    non-overlapping pair trick (1.5 ops/pixel).
  - The scalar (ACT) HWDGE ring sustains ~300-370 GB/s with 32 KiB
    descriptors; the sync (SP) ring only ~100-230 GB/s. Descriptor size is
    what matters (8 KiB descriptors halve the rate), so bulk transfers stay
    in >=16 KiB-per-partition chunks. Loads + the final store ride the
    scalar ring; mid-kernel stores ride the otherwise-idle sync ring.
  - Partition-shift halo rows come from PE shift-matmuls (shifted-identity
    weights built on-device with affine_select; loading them from DRAM
    would head-of-line block a DGE ring with 256 tiny descriptors) into
    PSUM: no DMA descriptors, and the shifted-out rows are zero = exactly
    the zero padding.
  - W-border max-with-0 runs as DVE tensor_scalar_max (keeping the ACT
    engine DMA-only avoids its 1.3 us activation-table load at startup).
  - Image 0's first half-load feeds DVE ~12 us in; the last image computes
    output rows 0-3 first so its final store is a single well-overlapped
    2 MiB transfer (16 KiB descriptors = full ring rate).
"""

import sys

sys.path.insert(0, "/opt/trn_rl_repo")

import numpy as np

N_CORES = 8
IMGS_PER_CORE = 3
H = W = 1024
R = 8  # rows per partition
P = 128

_COMPILED_NC = None


def _build_nc():
    import concourse.mybir as mybir
    import concourse.tile as tile
    from concourse import bacc

    f32 = mybir.dt.float32
    MAX = mybir.AluOpType.max

    nc = bacc.Bacc(None)
    x = nc.declare_dram_parameter("input", [IMGS_PER_CORE, H, W], f32, isOutput=False)
    y = nc.declare_dram_parameter("output", [IMGS_PER_CORE, H, W], f32, isOutput=True)

    J = W // 2  # 512 horizontal pairs per row
    Q = R // 2  # 4 vertical pairs per partition

    with tile.TileContext(nc) as tc:
        with (
            tc.tile_pool(name="io", bufs=3) as io,
            tc.tile_pool(name="hmp", bufs=2) as hmp,
            tc.tile_pool(name="hpp", bufs=1) as hpp,
            tc.tile_pool(name="vpp", bufs=1) as vpp,
            tc.tile_pool(name="shp", bufs=1) as shp,
            tc.tile_pool(name="psum", bufs=2, space="PSUM") as psp,
        ):
            # Shifted identities built on the (otherwise idle) Pool engine —
            # loading them from DRAM would head-of-line block a DGE ring with
            # 256 tiny descriptors.
            # sdn[k, m] = 1 iff k == m-1; sup[k, m] = 1 iff k == m+1
            sdn = shp.tile([P, P], f32, tag="sdn")
            sup = shp.tile([P, P], f32, tag="sup")
            for t, base in ((sdn, 1), (sup, -1)):
                nc.gpsimd.memset(t[:], 0.0)
                nc.gpsimd.affine_select(
                    out=t[:],
                    in_=t[:],
                    compare_op=mybir.AluOpType.not_equal,
                    fill=1.0,
                    base=base,
                    pattern=[[-1, P]],
                    channel_multiplier=1,
                )

            for i in range(IMGS_PER_CORE):
                xi = x[i].rearrange("(p r) w -> p (r w)", r=R)  # [128, 8192]
                yi = y[i].rearrange("(p r) w -> p (r w)", r=R)

                # 2 MiB half-loads: 16 KiB-per-partition descriptors keep the
                # rings at full rate (8 KiB descriptors drop to ~134 GB/s).
                # The very first chunk rides the sync ring, which comes up at
                # ~5 us vs the scalar ring's ~8.7 us, so DVE starts earliest.
                chunks = [(0, 4), (4, 8)]
                X = io.tile([P, R * W], f32, tag="io")
                for r0, r1 in chunks:
                    nc.scalar.dma_start(
                        out=X[:, r0 * W : r1 * W], in_=xi[:, r0 * W : r1 * W]
                    )
                X4 = X[:].rearrange("p (r j t) -> p r j t", j=J, t=2)

                # --- horizontal 3-max ---
                hp = hpp.tile([P, R * J], f32, tag="hp")
                hp3 = hp[:].rearrange("p (r j) -> p r j", j=J)
                hm = hmp.tile([P, R * W], f32, tag="hm")
                hm4 = hm[:].rearrange("p (r j t) -> p r j t", j=J, t=2)
                h_chunks = chunks if i == 0 else [(0, R)]
                for r0, r1 in h_chunks:
                    nc.vector.tensor_tensor(
                        out=hp3[:, r0:r1],
                        in0=X4[:, r0:r1, :, 0],
                        in1=X4[:, r0:r1, :, 1],
                        op=MAX,
                    )
                    # hmax[2j] = max(X[2j-1], hp[j]), j=1..511
                    nc.vector.tensor_tensor(
                        out=hm4[:, r0:r1, 1:J, 0],
                        in0=X4[:, r0:r1, 0 : J - 1, 1],
                        in1=hp3[:, r0:r1, 1:J],
                        op=MAX,
                    )
                    # hmax[2j+1] = max(hp[j], X[2j+2]), j=0..510
                    nc.vector.tensor_tensor(
                        out=hm4[:, r0:r1, 0 : J - 1, 1],
                        in0=hp3[:, r0:r1, 0 : J - 1],
                        in1=X4[:, r0:r1, 1:J, 0],
                        op=MAX,
                    )
                    # w-borders: max with the zero pad
                    nc.vector.tensor_scalar_max(
                        out=hm4[:, r0:r1, 0, 0], in0=hp3[:, r0:r1, 0], scalar1=0.0
                    )
                    nc.vector.tensor_scalar_max(
                        out=hm4[:, r0:r1, J - 1, 1],
                        in0=hp3[:, r0:r1, J - 1],
                        scalar1=0.0,
                    )

                # --- partition-boundary halo rows via PE shift-matmul ---
                # uh[p] = hmax[p+1, row 0] (row 0 ready first); dh[p] =
                # hmax[p-1, row 7]. Out-of-range rows are zero = the padding.
                dh = psp.tile([P, W], f32, tag="dh")
                uh = psp.tile([P, W], f32, tag="uh")
                last = (R - 1) * W
                for c0 in (0, 512):
                    nc.tensor.matmul(
                        uh[:, c0 : c0 + 512],
                        sup[:],
                        hm[:, c0 : c0 + 512],
                        start=True,
                        stop=True,
                    )
                for c0 in (0, 512):
                    nc.tensor.matmul(
                        dh[:, c0 : c0 + 512],
                        sdn[:],
                        hm[:, last + c0 : last + c0 + 512],
                        start=True,
                        stop=True,
                    )

                # --- vertical 3-max ---
                hm3 = hm[:].rearrange("p (r w) -> p r w", w=W)
                vp = vpp.tile([P, Q * W], f32, tag="vp")
                vp3 = vp[:].rearrange("p (q w) -> p q w", w=W)
                vm = io.tile([P, R * W], f32, tag="io")
                vm3 = vm[:].rearrange("p (r w) -> p r w", w=W)
                st = getattr(nc, "sync" if i < IMGS_PER_CORE - 1 else "scalar")

                # vp[q] = max(hmax[2q], hmax[2q+1])
                nc.vector.tensor_tensor(
                    out=vp3[:, 0:2], in0=hm3[:, 0:4:2], in1=hm3[:, 1:4:2], op=MAX
                )
                nc.vector.tensor_tensor(
                    out=vp3[:, 2:4], in0=hm3[:, 4:8:2], in1=hm3[:, 5:8:2], op=MAX
                )
                if i < IMGS_PER_CORE - 1:
                    # mid-kernel: fewest ops; stores hide under later compute
                    nc.vector.tensor_tensor(
                        out=vm3[:, 1:4:2], in0=vp3[:, 0:2], in1=hm3[:, 2:6:2], op=MAX
                    )
                    nc.vector.tensor_tensor(
                        out=vm3[:, 2:8:2], in0=hm3[:, 1:7:2], in1=vp3[:, 1:4], op=MAX
                    )
                    nc.vector.tensor_tensor(
                        out=vm3[:, 0], in0=dh[:, :], in1=vp3[:, 0], op=MAX
                    )
                    nc.vector.tensor_tensor(
                        out=vm3[:, 5], in0=vp3[:, 2], in1=hm3[:, 6], op=MAX
                    )
                    nc.vector.tensor_tensor(
                        out=vm3[:, 7], in0=vp3[:, 3], in1=uh[:, :], op=MAX
                    )
                    st.dma_start(out=yi[:, 0 : 4 * W], in_=vm[:, 0 : 4 * W])
                    st.dma_start(out=yi[:, 4 * W : 8 * W], in_=vm[:, 4 * W : 8 * W])
                else:
                    # last image: finish rows 0-3 first; the final 2 MiB store
                    # (16 KiB descriptors - full ring rate) starts right after
                    # the last DVE op.
                    nc.vector.tensor_tensor(
                        out=vm3[:, 1:4:2], in0=vp3[:, 0:2], in1=hm3[:, 2:6:2], op=MAX
                    )
                    nc.vector.tensor_tensor(
                        out=vm3[:, 2], in0=hm3[:, 1], in1=vp3[:, 1], op=MAX
                    )
                    nc.vector.tensor_tensor(
                        out=vm3[:, 0], in0=dh[:, :], in1=vp3[:, 0], op=MAX
                    )
                    st.dma_start(out=yi[:, 0 : 4 * W], in_=vm[:, 0 : 4 * W])
                    nc.vector.tensor_tensor(
                        out=vm3[:, 4:8:2], in0=hm3[:, 3:7:2], in1=vp3[:, 2:4], op=MAX
                    )
                    nc.vector.tensor_tensor(
                        out=vm3[:, 5], in0=vp3[:, 2], in1=hm3[:, 6], op=MAX
                    )
                    nc.vector.tensor_tensor(
                        out=vm3[:, 7], in0=vp3[:, 3], in1=uh[:, :], op=MAX
                    )
                    st.dma_start(out=yi[:, 4 * W : 8 * W], in_=vm[:, 4 * W : 8 * W])

    nc.compile()
    return nc


def _get_nc():
    global _COMPILED_NC
    if _COMPILED_NC is None:
        _COMPILED_NC = _build_nc()
    return _COMPILED_NC


def _reference_fallback(input, se):
    # Generic path (never hit for the graded all-ones 3x3 se); mirrors the
    # kornia Dilate reference exactly.
    se = np.asarray(se, dtype=np.float32)
    se_h, se_w = se.shape
    pad_h, pad_w = se_h // 2, se_w // 2
    B, C, Hh, Ww = input.shape
    se_m1 = (se - 1.0).reshape(-1)
    padded = np.pad(input, ((0, 0), (0, 0), (pad_h, pad_h), (pad_w, pad_w)))
    out = None
    for i in range(se_h * se_w):
        xs, ys = i // se_h, i % se_h
        mask = np.float32(1.0) if se_m1[i] >= 0 else np.float32(0.0)
        contrib = mask * padded[:, :, xs : xs + Hh, ys : ys + Ww] + se_m1[i]
        out = contrib if out is None else np.maximum(out, contrib)
    return out


def kernel(input, se):
    from concourse.bass_utils import run_bass_kernel_spmd

    input = np.ascontiguousarray(np.asarray(input, dtype=np.float32))
    se_np = np.asarray(se, dtype=np.float32)
    if se_np.shape != (3, 3) or not np.all(se_np == 1.0) or input.shape != (
        8,
        3,
        H,
        W,
    ):
        return _reference_fallback(input, se_np).astype(np.float32)

    nc = _get_nc()
    flat = input.reshape(N_CORES * IMGS_PER_CORE, H, W)
    in_maps = [
        {"input": flat[k * IMGS_PER_CORE : (k + 1) * IMGS_PER_CORE]}
        for k in range(N_CORES)
    ]
    last_err = None
    for _attempt in range(3):
        try:
            res = run_bass_kernel_spmd(nc, in_maps, list(range(N_CORES)))
            out = np.concatenate(
                [res.results[k]["output"] for k in range(N_CORES)], axis=0
            )
            return out.reshape(8, 3, H, W)
        except Exception as e:  # transient NRT_EXEC_UNIT_UNRECOVERABLE etc.
            last_err = e
    raise last_err



# revision 2
# speedup vs baseline: 1.0226x; 1.0226x over previous
"""3x3 grayscale dilation (all-ones SE) = 3x3 max-pool, stride 1, zero padding.

Input (8, 3, 1024, 1024) f32 -> same-shape output.
Sharding: 24 (B*C) images, 3 per NeuronCore across 8 cores.

Measured HW model this design is built on (all measured on these trn2 cores):
  - DVE fp32 tensor_tensor: ~0.95-1.1 elem/ns regardless of striding; fp16
    tensor_tensor with uniform-phase packed access: ~1.9 elem/ns; fp16 with
    mixed 2-byte phase or stride-2 inner: ~0.75 (so the H-pass +-1-element
    shifts must stay fp32, and fp16 tiles must be accessed at 4B-aligned
    offsets only).
  - Mixed-dtype TT (fp32 ins -> fp16 out) runs at the fp32 rate: the H-pass
    downcast is free.
  - gpsimd rejects tensor_tensor; DVE pool fails ISA checks; tensor_reduce
    runs at ~1 read/ns (no help). ACT copy PSUM->SBUF fp16 (512-1024 wide)
    is ~0.34us: halo evacuation is nearly free on the idle scalar engine.

Plan per image ([128, 8192] layout, partition p = image rows 8p..8p+7):
  H-pass (fp32, stride-2 pairing, 1.5 ops/px):
    hp[j] = max(x[2j], x[2j+1]) -> fp32 [r, 512]
    hm_even[j] = max(x[2j-1], hp[j]); hm_odd[j] = max(hp[j], x[2j+2])
    written as fp16 into an even/odd-SPLIT tile (contiguous halves keep
    fp16 phase uniform), with w-border columns maxed against the 0 pad.
  HMEO tile [128, 2x10x512] fp16: per half a in {even, odd}: slot 0 = down
    halo (hm[p-1, row7]), slots 1..8 = rows 0..7, slot 9 = up halo.
    Halos via PE shift-matmuls (fp16 shifted identities) into PSUM,
    evacuated by ACT copies into the halo slots.
  V-pass (all fp16 at ~1.9 elem/ns, pairing, both halves in one op each):
    vp[a,q] = max(slot 2q+1, slot 2q+2)
    vm rows 1,3,5 = max(vp[q], slot 2q+3); rows 2,4,6 = max(slot 2q, vp[q])
    row 0 = max(slot0, vp[0]); row 7 = max(vp[3], slot9)
  VM tile [128, 8x1024] fp16, each row = [512 evens | 512 odds]; stored to
  DRAM fp16; the host de-interleaves and upcasts to fp32 (equivalent to
  doing the same on-device; max-pool only selects values, so the only error
  is the fp16 round of the selected value, ~2^-11 rel, vs the 2e-2 gate).
"""

import sys

sys.path.insert(0, "/opt/trn_rl_repo")

import numpy as np

N_CORES = 8
IMGS_PER_CORE = 3
H = W = 1024
R = 8  # rows per partition
P = 128
J = W // 2  # 512

_COMPILED_NC = None


def _build_nc():
    import concourse.mybir as mybir
    import concourse.tile as tile
    from concourse import bacc

    f16 = mybir.dt.float16
    f32 = mybir.dt.float32
    MAX = mybir.AluOpType.max

    nc = bacc.Bacc(None)
    x = nc.declare_dram_parameter("input", [IMGS_PER_CORE, H, W], f32, isOutput=False)
    y = nc.declare_dram_parameter("output", [IMGS_PER_CORE, H, W], f16, isOutput=True)

    NW = R * W  # 8192

    with tile.TileContext(nc) as tc:
        with (
            tc.tile_pool(name="xp", bufs=3) as xp,
            tc.tile_pool(name="hpp", bufs=1) as hpp,
            tc.tile_pool(name="hmp", bufs=2) as hmp,
            tc.tile_pool(name="vpp", bufs=1) as vpp,
            tc.tile_pool(name="vmp", bufs=2) as vmp,
            tc.tile_pool(name="shp", bufs=1) as shp,
            tc.tile_pool(name="psum", bufs=2, space="PSUM") as psp,
        ):
            # fp16 shifted identities for the PE halo shifts.
            # sdn[k, m] = 1 iff k == m-1 (out[m] = in[m-1] = down-halo source)
            # sup[k, m] = 1 iff k == m+1 (out[m] = in[m+1])
            sdn = shp.tile([P, P], f16, tag="sdn")
            sup = shp.tile([P, P], f16, tag="sup")
            for t, base in ((sdn, 1), (sup, -1)):
                nc.gpsimd.memset(t[:], 0.0)
                nc.gpsimd.affine_select(
                    out=t[:],
                    in_=t[:],
                    compare_op=mybir.AluOpType.not_equal,
                    fill=1.0,
                    base=base,
                    pattern=[[-1, P]],
                    channel_multiplier=1,
                )

            xs = []
            for i in range(IMGS_PER_CORE):
                xi = x[i].rearrange("(p r) w -> p (r w)", r=R)
                X = xp.tile([P, NW], f32, tag="x")
                # all loads on the scalar ring (~430 GB/s measured; the sync
                # ring only ~190). Image 0 in halves so the H-pass can start
                # after the first half lands.
                if i == 0:
                    nc.scalar.dma_start(out=X[:, 0 : NW // 2], in_=xi[:, 0 : NW // 2])
                    nc.scalar.dma_start(out=X[:, NW // 2 : NW], in_=xi[:, NW // 2 : NW])
                else:
                    nc.scalar.dma_start(out=X[:], in_=xi[:])
                xs.append(X)

            for i in range(IMGS_PER_CORE):
                yi = y[i].rearrange("(p r) w -> p (r w)", r=R)
                X = xs[i]
                X4 = X[:].rearrange("p (r j t) -> p r j t", j=J, t=2)

                HP = hpp.tile([P, R * J], f32, tag="hp")
                HP3 = HP[:].rearrange("p (r j) -> p r j", j=J)
                HM = hmp.tile([P, 2 * 10 * J], f16, tag="hm")
                HM4 = HM[:].rearrange("p (a s j) -> p a s j", a=2, s=10, j=J)

                # --- horizontal pass (fp32 pairing, fp16 split outs) ---
                chunks = [(0, 4), (4, R)] if i == 0 else [(0, R)]
                for r0, r1 in chunks:
                    s0, s1 = r0 + 1, r1 + 1  # hm slots for these rows
                    nc.vector.tensor_tensor(
                        out=HP3[:, r0:r1],
                        in0=X4[:, r0:r1, :, 0],
                        in1=X4[:, r0:r1, :, 1],
                        op=MAX,
                    )
                    # even outs: hm_e[j] = max(x[2j-1], hp[j]), j=1..511
                    nc.vector.tensor_tensor(
                        out=HM4[:, 0, s0:s1, 1:J],
                        in0=X4[:, r0:r1, 0 : J - 1, 1],
                        in1=HP3[:, r0:r1, 1:J],
                        op=MAX,
                    )
                    # odd outs: hm_o[j] = max(hp[j], x[2j+2]), j=0..510
                    nc.vector.tensor_tensor(
                        out=HM4[:, 1, s0:s1, 0 : J - 1],
                        in0=HP3[:, r0:r1, 0 : J - 1],
                        in1=X4[:, r0:r1, 1:J, 0],
                        op=MAX,
                    )
                    # w-borders: max with the zero pad
                    nc.vector.tensor_scalar_max(
                        out=HM4[:, 0, s0:s1, 0], in0=HP3[:, r0:r1, 0], scalar1=0.0
                    )
                    nc.vector.tensor_scalar_max(
                        out=HM4[:, 1, s0:s1, J - 1],
                        in0=HP3[:, r0:r1, J - 1],
                        scalar1=0.0,
                    )

                # --- partition halos via PE shift-matmuls (fp16) ---
                dhE = psp.tile([P, J], f32, tag="dhE")
                dhO = psp.tile([P, J], f32, tag="dhO")
                uhE = psp.tile([P, J], f32, tag="uhE")
                uhO = psp.tile([P, J], f32, tag="uhO")
                # down-halo: hm[p-1, row7] = slot 8; up-halo: hm[p+1, row0] = slot 1
                nc.tensor.matmul(dhE[:], sdn[:], HM4[:, 0, 8, :], start=True, stop=True)
                nc.tensor.matmul(dhO[:], sdn[:], HM4[:, 1, 8, :], start=True, stop=True)
                nc.tensor.matmul(uhE[:], sup[:], HM4[:, 0, 1, :], start=True, stop=True)
                nc.tensor.matmul(uhO[:], sup[:], HM4[:, 1, 1, :], start=True, stop=True)
                # ACT evacuates PSUM into the fp16 halo slots (~0.34us each)
                nc.scalar.copy(out=HM4[:, 0, 0, :], in_=dhE[:])
                nc.scalar.copy(out=HM4[:, 1, 0, :], in_=dhO[:])
                nc.scalar.copy(out=HM4[:, 0, 9, :], in_=uhE[:])
                nc.scalar.copy(out=HM4[:, 1, 9, :], in_=uhO[:])

                # --- vertical pass (fp16, both halves per op) ---
                VP = vpp.tile([P, 2 * 4 * J], f16, tag="vp")
                VP3 = VP[:].rearrange("p (a q j) -> p a q j", a=2, q=4, j=J)
                VM = vmp.tile([P, NW], f16, tag="vm")
                VM4 = VM[:].rearrange("p (r a j) -> p a r j", r=R, a=2, j=J)

                # vp[a,q] = max(slot 2q+1, slot 2q+2)  (rows 2q, 2q+1)
                nc.vector.tensor_tensor(
                    out=VP3[:], in0=HM4[:, :, 1:9:2, :], in1=HM4[:, :, 2:10:2, :], op=MAX
                )
                last = i == IMGS_PER_CORE - 1
                if not last:
                    # rows 1,3,5 = max(vp[q], slot 2q+3)
                    nc.vector.tensor_tensor(
                        out=VM4[:, :, 1:7:2, :],
                        in0=VP3[:, :, 0:3, :],
                        in1=HM4[:, :, 3:9:2, :],
                        op=MAX,
                    )
                    # rows 2,4,6 = max(slot 2q, vp[q]), q=1..3
                    nc.vector.tensor_tensor(
                        out=VM4[:, :, 2:8:2, :],
                        in0=HM4[:, :, 2:8:2, :],
                        in1=VP3[:, :, 1:4, :],
                        op=MAX,
                    )
                    # row 0 = max(down-halo, vp[0]); row 7 = max(vp[3], up-halo)
                    nc.vector.tensor_tensor(
                        out=VM4[:, :, 0, :], in0=HM4[:, :, 0, :], in1=VP3[:, :, 0, :], op=MAX
                    )
                    nc.vector.tensor_tensor(
                        out=VM4[:, :, R - 1, :],
                        in0=VP3[:, :, 3, :],
                        in1=HM4[:, :, 9, :],
                        op=MAX,
                    )
                    nc.sync.dma_start(out=yi[:], in_=VM[:])
                else:
                    # finish rows 0-3 first; store them early on the scalar
                    # ring while rows 4-7 finish, which then ride the gpsimd
                    # ring so both stores drain in parallel.
                    nc.vector.tensor_tensor(
                        out=VM4[:, :, 1:5:2, :],
                        in0=VP3[:, :, 0:2, :],
                        in1=HM4[:, :, 3:7:2, :],
                        op=MAX,
                    )
                    nc.vector.tensor_tensor(
                        out=VM4[:, :, 2, :], in0=HM4[:, :, 2, :], in1=VP3[:, :, 1, :], op=MAX
                    )
                    nc.vector.tensor_tensor(
                        out=VM4[:, :, 0, :], in0=HM4[:, :, 0, :], in1=VP3[:, :, 0, :], op=MAX
                    )
                    nc.scalar.dma_start(out=yi[:, 0 : NW // 2], in_=VM[:, 0 : NW // 2])
                    nc.vector.tensor_tensor(
                        out=VM4[:, :, 5, :], in0=VP3[:, :, 2, :], in1=HM4[:, :, 7, :], op=MAX
                    )
                    nc.vector.tensor_tensor(
                        out=VM4[:, :, 4:8:2, :],
                        in0=HM4[:, :, 4:8:2, :],
                        in1=VP3[:, :, 2:4, :],
                        op=MAX,
                    )
                    nc.vector.tensor_tensor(
                        out=VM4[:, :, R - 1, :],
                        in0=VP3[:, :, 3, :],
                        in1=HM4[:, :, 9, :],
                        op=MAX,
                    )
                    nc.gpsimd.dma_start(out=yi[:, NW // 2 : NW], in_=VM[:, NW // 2 : NW])

    nc.compile()
    return nc


def _get_nc():
    global _COMPILED_NC
    if _COMPILED_NC is None:
        _COMPILED_NC = _build_nc()
    return _COMPILED_NC


def _reference_fallback(input, se):
    # Generic path (never hit for the graded all-ones 3x3 se); mirrors the
    # kornia Dilate reference exactly.
    se = np.asarray(se, dtype=np.float32)
    se_h, se_w = se.shape
    pad_h, pad_w = se_h // 2, se_w // 2
    B, C, Hh, Ww = input.shape
    se_m1 = (se - 1.0).reshape(-1)
    padded = np.pad(input, ((0, 0), (0, 0), (pad_h, pad_h), (pad_w, pad_w)))
    out = None
    for i in range(se_h * se_w):
        xs, ys = i // se_h, i % se_h
        mask = np.float32(1.0) if se_m1[i] >= 0 else np.float32(0.0)
        contrib = mask * padded[:, :, xs : xs + Hh, ys : ys + Ww] + se_m1[i]
        out = contrib if out is None else np.maximum(out, contrib)
    return out


def kernel(input, se):
    from concourse.bass_utils import run_bass_kernel_spmd

    input = np.asarray(input)
    se_np = np.asarray(se, dtype=np.float32)
    if se_np.shape != (3, 3) or not np.all(se_np == 1.0) or input.shape != (
        8,
        3,
        H,
        W,
    ):
        return _reference_fallback(
            np.ascontiguousarray(input, dtype=np.float32), se_np
        ).astype(np.float32)

    nc = _get_nc()
    flat = np.ascontiguousarray(input, dtype=np.float32).reshape(
        N_CORES * IMGS_PER_CORE, H, W
    )
    in_maps = [
        {"input": flat[k * IMGS_PER_CORE : (k + 1) * IMGS_PER_CORE]}
        for k in range(N_CORES)
    ]
    last_err = None
    for _attempt in range(3):
        try:
            res = run_bass_kernel_spmd(nc, in_maps, list(range(N_CORES)))
            out16 = np.concatenate(
                [res.results[k]["output"] for k in range(N_CORES)], axis=0
            )  # [24, 1024, 1024] fp16, rows = [512 evens | 512 odds]
            out = (
                out16.reshape(N_CORES * IMGS_PER_CORE, H, 2, J)
                .transpose(0, 1, 3, 2)
                .reshape(N_CORES * IMGS_PER_CORE, H, W)
                .astype(np.float32)
            )
            return out.reshape(8, 3, H, W)
        except Exception as e:  # transient NRT_EXEC_UNIT_UNRECOVERABLE etc.
            last_err = e
    raise last_err
